# revision 1
# baseline (speedup 1.0000x reference)
"""Trainium2 Bass kernel for nn_CausalSelfAttention_8237747274097.

Reference math (single-head attention over full n_embd=1024, scale 1/8):
    qkv = x @ W_attn + b_attn ; q,k,v = split(qkv)
    att = softmax(causal(q @ k.T / 8)) ; y = att @ v ; out = y @ W_proj + b_proj

Sharding (8 cores): core c = (batch b = c//2, parity p = c%2). Each core owns 8
of the 16 query row-tiles (128 rows each) of its batch, interleaved/paired so
causal work is balanced, and computes full K/V for the batch. Outputs are
disjoint row slices -> host gather is a pure scatter + bias add.

Math simplifications (all exact):
  - k bias drops out of softmax (constant along the softmax axis after the
    q.bias cross term is absorbed; verified exact in float64).
  - v bias folds into the output bias: b_eff = b_proj + b_v @ W_proj.
  - 1/8 scale folded into W_q/b_q on the host (exact power of two).
Softmax is computed without max-subtraction (scores are O(3); exp is safe) so
the denominator comes free from a ones-row matmul.

Precision: fp32 storage with float32r matmuls (full PE rate); P=exp(S) and V
are bf16 (value-side rounding only; num/den share the same rounded P).
"""

import numpy as np
import ml_dtypes

import concourse.bass as bass
import concourse.tile as tile
import concourse.mybir as mybir
from concourse import bacc
from concourse.bass import ts, ds
from concourse.bass_utils import run_bass_kernel_spmd

F32 = mybir.dt.float32
F32R = mybir.dt.float32r
BF16 = mybir.dt.bfloat16
F16 = mybir.dt.float16

T, D = 2048, 1024
NT = T // 128          # 16 query/key tiles
DC = D // 128          # 8 contraction chunks
# own query tiles per core parity (descending pairing balances causal work)
OWN = [[15, 12, 11, 8, 7, 4, 3, 0],
       [14, 13, 10, 9, 6, 5, 2, 1]]
CP = [16, 12, 8, 4]    # j-blocks computed per slot-pair (uniform across cores)

_NC_CACHE = {}


def _build(repeat=1, phases=3):
    key = (repeat, phases)
    if key in _NC_CACHE:
        return _NC_CACHE[key]
    nc = bacc.Bacc("TRN2", target_bir_lowering=False, debug=False,
                   enable_asserts=False, num_devices=8)
    xT = nc.dram_tensor("xT", [D, T], F32R, kind="ExternalInput").ap()
    xqT = nc.dram_tensor("xqT", [D, 1024], F32R, kind="ExternalInput").ap()
    wq = nc.dram_tensor("wq", [D, D], F32R, kind="ExternalInput").ap()
    wk = nc.dram_tensor("wk", [D, D], F32R, kind="ExternalInput").ap()
    wv = nc.dram_tensor("wv", [D, D], F32R, kind="ExternalInput").ap()
    wp = nc.dram_tensor("wp", [D, D], F32R, kind="ExternalInput").ap()
    bq = nc.dram_tensor("bq", [D], F32, kind="ExternalInput").ap()
    masks = nc.dram_tensor("masks", [16, 128, 256], F16, kind="ExternalInput").ap()
    out = nc.dram_tensor("out", [1024, D], F32, kind="ExternalOutput").ap()
    den_dram = nc.dram_tensor("den_scratch", [1024], F32).ap()

    with tile.TileContext(nc, pool_alloc_mode="queue") as tc:
        def body(_i=None):
            _emit(nc, tc, xT, xqT, wq, wk, wv, wp, bq, masks, out, den_dram, phases)
        if repeat == 1:
            body()
        else:
            with tc.For_i(0, repeat, 1):
                body()
    nc.compile()
    _NC_CACHE[key] = nc
    return nc


def _emit(nc, tc, xT, xqT, wq, wk, wv, wp, bq, masks, out, den_dram, phases=3):
    with tc.tile_pool(name="pk", bufs=1) as pk_pool, \
         tc.tile_pool(name="pv", bufs=1) as pv_pool, \
         tc.tile_pool(name="pq", bufs=1) as pq_pool, \
         tc.tile_pool(name="small", bufs=1) as small:

        # ---------------- Phase A1: K^T (needs xt) ----------------
        with tc.tile_pool(name="xt", bufs=1) as xt_pool:
            xt = [[xt_pool.tile([128, 512], F32R, tag=f"xt{d}_{j}", name=f"xt{d}_{j}")
                   for j in range(4)] for d in range(DC)]
            for d in range(DC):
                for j in range(4):
                    nc.sync.dma_start(xt[d][j][:], xT[ts(d, 128), ts(j, 512)])

            kT_sb = [pk_pool.tile([128, T], F16, tag=f"k{m}", name=f"kT_sb{m}") for m in range(DC)]
            with tc.tile_pool(name="wkm", bufs=2) as wk_pool, \
                 tc.tile_pool(name="psA1", bufs=2, space="PSUM") as psA1:
                for m in range(DC):
                    wkm = wk_pool.tile([128, 1024], F32R, tag="wkm", name="wkm")
                    nc.sync.dma_start(
                        wkm[:].rearrange("p (c f) -> p c f", f=128),
                        wk[:, ts(m, 128)].rearrange("(c p) f -> p c f", p=128))
                    ps = psA1.tile([128, 2048], F32, tag="A1", name="psA1_t")
                    for jc in range(4):
                        for d in range(DC):
                            nc.tensor.matmul(ps[:, ts(jc, 512)],
                                             wkm[:, ts(d, 128)],
                                             xt[d][jc][:],
                                             start=(d == 0), stop=(d == DC - 1))
                    nc.scalar.copy(kT_sb[m][:], ps[:])

            # ---------------- Phase A0: V (wv streamed in halves) ----------------
            v_sb = [pv_pool.tile([128, D], F16, tag=f"v{t}", name=f"v_sb{t}") for t in range(NT)]
            with tc.tile_pool(name="wv", bufs=1) as wv_pool, \
                 tc.tile_pool(name="psA", bufs=4, space="PSUM") as psA:
                for fc in range(2):
                    wv_sb = [wv_pool.tile([128, 512], F32R, tag=f"wv{d}", name=f"wv_sb{d}")
                             for d in range(DC)]
                    for d in range(DC):
                        nc.sync.dma_start(wv_sb[d][:], wv[ts(d, 128), ts(fc, 512)])
                    for tt in range(NT):
                        ps = psA.tile([128, 512], F32, tag="A", name="psA_t")
                        for d in range(DC):
                            nc.tensor.matmul(ps[:],
                                             xt[d][tt // 4][:, ts(tt % 4, 128)],
                                             wv_sb[d][:],
                                             start=(d == 0), stop=(d == DC - 1))
                        nc.vector.tensor_copy(v_sb[tt][:, ts(fc, 512)], ps[:])

        # ---------------- Phase A2: Q^T (own rows; xt freed, xq loaded) ----------------
        bq_sb = small.tile([128, 8], F32, tag="bq", name="bq_sb")
        nc.sync.dma_start(bq_sb[:], bq.rearrange("(m p) -> p m", p=128))
        qT_sb = [[pq_pool.tile([128, 256], F16, tag=f"q{m}_{p}", name=f"qT_sb{m}_{p}")
                  for p in range(4)] for m in range(DC)]
        with tc.tile_pool(name="xq", bufs=1) as xq_pool:
            xq = [[xq_pool.tile([128, 512], F32R, tag=f"xq{d}_{j}", name=f"xq{d}_{j}")
                   for j in range(2)] for d in range(DC)]
            for d in range(DC):
                for j in range(2):
                    nc.sync.dma_start(xq[d][j][:], xqT[ts(d, 128), ts(j, 512)])
            with tc.tile_pool(name="wqm", bufs=2) as wq_pool, \
                 tc.tile_pool(name="psA2", bufs=2, space="PSUM") as psA2:
                for m in range(DC):
                    wqm = wq_pool.tile([128, 1024], F32R, tag="wqm", name="wqm")
                    nc.sync.dma_start(
                        wqm[:].rearrange("p (c f) -> p c f", f=128),
                        wq[:, ts(m, 128)].rearrange("(c p) f -> p c f", p=128))
                    ps = psA2.tile([128, 1024], F32, tag="A2", name="psA2_t")
                    for ic in range(2):
                        for d in range(DC):
                            nc.tensor.matmul(ps[:, ts(ic, 512)],
                                             wqm[:, ts(d, 128)],
                                             xq[d][ic][:],
                                             start=(d == 0), stop=(d == DC - 1))
                    for p in range(4):
                        nc.scalar.activation(qT_sb[m][p][:], ps[:, ts(p, 256)],
                                             mybir.ActivationFunctionType.Identity,
                                             bias=bq_sb[:, m:m + 1])

        if phases <= 1:
            # keep outputs alive: dump a token result so nothing is DCE'd
            with tc.tile_pool(name="dump", bufs=1) as dump:
                tk = dump.tile([128, 512], F32, tag="tk", name="tk")
                nc.vector.tensor_copy(tk[:], kT_sb[0][:, 0:512])
                nc.sync.dma_start(out[0:128, 0:512], tk[:])
                tq = dump.tile([128, 512], F32, tag="tq", name="tq")
                nc.vector.tensor_copy(tq[:, 0:256], qT_sb[0][0][:])
                nc.sync.dma_start(out[0:128, 512:1024], tq[:])
                tv = dump.tile([128, 512], F32, tag="tv", name="tv")
                nc.vector.tensor_copy(tv[:], v_sb[0][:, 0:512])
                nc.sync.dma_start(out[128:256, 0:512], tv[:])
            return

        # ---------------- Phase B: attention + projection ----------------
        with tc.tile_pool(name="transB", bufs=3) as trans, \
             tc.tile_pool(name="wp", bufs=1) as wp_pool, \
         tc.tile_pool(name="po", bufs=1, space="PSUM") as po_pool, \
             tc.tile_pool(name="psS", bufs=2, space="PSUM") as psS_pool, \
             tc.tile_pool(name="pden", bufs=1, space="PSUM") as pden_pool, \
             tc.tile_pool(name="pproj", bufs=1, space="PSUM") as pp_pool:

            ones_bf = small.tile([128, 1], F16, tag="ones", name="ones_bf")
            nc.vector.memset(ones_bf[:], 1.0)
            wp_sb = [[wp_pool.tile([128, 512], F32R, tag=f"wp{d}_{f}", name=f"wp_sb{d}_{f}")
                      for f in range(2)] for d in range(DC)]
            for d in range(DC):
                for f in range(2):
                    nc.sync.dma_start(wp_sb[d][f][:], wp[ts(d, 128), ts(f, 512)])

            for P in range(4):
                cp = CP[P]
                pden = pden_pool.tile([1, 256], F32, tag="den", name="pden_t")
                pts = []
                for tj in range(cp):
                    psS = psS_pool.tile([128, 256], F32, tag="s", name="psS_t")
                    for d in range(DC):
                        nc.tensor.matmul(psS[:],
                                         kT_sb[d][:, ts(tj, 128)],
                                         qT_sb[d][P][:],
                                         start=(d == 0), stop=(d == DC - 1))
                    pt = trans.tile([128, 256], F16, tag=f"pt{tj}", name="pt_t", bufs=2)
                    nc.scalar.activation(pt[:], psS[:],
                                         mybir.ActivationFunctionType.Exp)
                    mi = tj - (cp - 4)
                    if mi >= 0:
                        mt = trans.tile([128, 256], F16, tag="mask", name="mt_t")
                        nc.sync.dma_start(mt[:], masks[4 * P + mi, :, :])
                        nc.vector.tensor_mul(pt[:], pt[:], mt[:])
                    pts.append(pt)
                    nc.tensor.matmul(pden[:], ones_bf[:], pt[:],
                                     start=(tj == 0), stop=(tj == cp - 1))

                if phases <= 2:
                    den_row0 = trans.tile([1, 256], F32, tag="denrow", name="den_row0")
                    nc.vector.tensor_copy(den_row0[:], pden[:])
                    nc.sync.dma_start(out[ds(128 * (2 * P), 1), ds(0, 256)], den_row0[:])
                    continue

                oT = trans.tile([128, 2048], F32R, tag="oT", name="oT_t", bufs=2)
                # PV in two d-halves: each accumulation group owns a full PSUM
                # bank (start=True clears has_written bank-wide).
                for half in range(2):
                    po = po_pool.tile([128, 2048], F32, tag="o", name="po_t")
                    for tj in range(cp):
                        for dtl in range(4):
                            dt = 4 * half + dtl
                            nc.tensor.matmul(po[:, ds(512 * dtl, 256)],
                                             v_sb[tj][:, ts(dt, 128)],
                                             pts[tj][:],
                                             start=(tj == 0), stop=(tj == cp - 1))
                    nc.vector.tensor_copy(
                        oT[:, ds(1024 * half, 1024)].rearrange("p (l x) -> p l x", x=256),
                        po[:].rearrange("p (l x) -> p l x", x=512)[:, :, 0:256])

                # denominator -> per-partition reciprocal columns
                den_row = trans.tile([1, 256], F32, tag="denrow", name="den_row")
                nc.vector.tensor_copy(den_row[:], pden[:])
                nc.sync.dma_start(den_dram[ds(256 * P, 256)], den_row[:])
                den_col = trans.tile([128, 2], F32, tag="dencol", name="den_col")
                nc.sync.dma_start(den_col[:],
                                  den_dram[ds(256 * P, 256)].rearrange("(t p) -> p t", p=128))
                recip = trans.tile([128, 2], F32, tag="recip", name="recip")
                nc.vector.reciprocal(recip[:], den_col[:])

                for it in range(2):
                    for fo in range(2):
                        pp = pp_pool.tile([128, 512], F32, tag="pp", name="pp_t")
                        for dt in range(DC):
                            nc.tensor.matmul(pp[:],
                                             oT[:, ds(256 * dt + 128 * it, 128)],
                                             wp_sb[dt][fo][:],
                                             start=(dt == 0), stop=(dt == DC - 1))
                        ob = trans.tile([128, 512], F32, tag="ob", name="ob_t")
                        nc.vector.tensor_scalar_mul(ob[:], pp[:], recip[:, it:it + 1])
                        nc.sync.dma_start(out[ds(128 * (2 * P + it), 128), ts(fo, 512)],
                                          ob[:])


def _host_masks(own):
    """(16, 128, 256) bf16 multiplicative masks for the last 4 tj of each pair."""
    m = np.zeros((16, 128, 256), np.float32)
    j = np.arange(128)[:, None]
    i = np.arange(128)[None, :]
    for P in range(4):
        cp = CP[P]
        for mi in range(4):
            tj = cp - 4 + mi
            for s in range(2):
                t = own[2 * P + s]
                m[4 * P + mi, :, 128 * s:128 * (s + 1)] = \
                    (128 * tj + j <= 128 * t + i).astype(np.float32)
    return m.astype(np.float16)


def kernel(x, W_attn, b_attn, W_proj, b_proj, _repeat=1, _results_only=False, _phases=3):
    x = np.asarray(x, np.float32)
    W_attn = np.asarray(W_attn, np.float32)
    b_attn = np.asarray(b_attn, np.float32)
    W_proj = np.asarray(W_proj, np.float32)
    b_proj = np.asarray(b_proj, np.float32)
    B = x.shape[0]

    nc = _build(_repeat, _phases)

    b_eff = (b_proj.astype(np.float64)
             + b_attn[2 * D:].astype(np.float64) @ W_proj.astype(np.float64)
             ).astype(np.float32)
    wq = np.ascontiguousarray(W_attn[:, :D]) * np.float32(0.125)
    wk = np.ascontiguousarray(W_attn[:, D:2 * D])
    wv = np.ascontiguousarray(W_attn[:, 2 * D:])
    bqv = b_attn[:D] * np.float32(0.125)
    masks_by_par = [_host_masks(OWN[0]), _host_masks(OWN[1])]

    in_maps = []
    for c in range(8):
        b, par = c // 2, c % 2
        own = OWN[par]
        xTb = np.ascontiguousarray(x[b].T)
        cols = np.concatenate([np.arange(128 * t, 128 * (t + 1)) for t in own])
        xqT = np.ascontiguousarray(xTb[:, cols])
        in_maps.append({"xT": xTb, "xqT": xqT, "wq": wq, "wk": wk, "wv": wv,
                        "wp": W_proj, "bq": bqv, "masks": masks_by_par[par]})

    res = run_bass_kernel_spmd(nc, in_maps, core_ids=list(range(8)))
    if _results_only:
        return res

    out = np.empty((B, T, D), np.float32)
    for c in range(8):
        b, par = c // 2, c % 2
        part = res.results[c]["out"]
        for s, t in enumerate(OWN[par]):
            out[b, 128 * t:128 * (t + 1), :] = part[128 * s:128 * (s + 1), :] + b_eff
    return out

